# revision 14
# baseline (speedup 1.0000x reference)
"""Trainium2 Bass kernel for nn_Closs_58978490909000.

Reference computes, per row r of f (8192 x 2048), with half = 1024:
    l_r = sum(f[r, half:]) - sum(f[r, :half])
          + sum_{i=0}^{half-1} [ logsumexp(f[r, i:N-i]) + logsumexp(-f[r, i:N-i]) ]
and returns mean_r l_r.

Algorithm (O(N) per row instead of O(N^2/2)):
  The windows are nested, so windowed sums of e = exp(+-f) obey a center-out
  additive recurrence.  A single DVE tensor_tensor_scan per row-tile computes
  all 1024 window sums:
      state_t = (e[1023-t] + state_{t-1}) + e[1024+t]     (t = 0..1023)
  (data0 = low half read with stride -1, data1 = high half forward), giving
  state_t = S_{1023-t}.  Then
      sum_i [ln S+_i + ln S-_i] = sum_i ln(S+_i * S-_i)
  so one elementwise product (on GPSIMD) halves the ACT ln work, and one Ln
  activation with fused accum_out produces the per-partition total.  No
  max-subtraction is needed: f ~ N(0,1) so exp() stays in [e^-6, e^6].

Engine budget per core (8 row-tiles of 128 rows x 2048):
  ACT   : 8 pair-batched exps ([128,4096] fp32->bf16) + 1 giant Ln [128,8192]
  DVE   : 16 scans (bf16, ~2.26us each) -- nothing else
  POOL  : 8 bf16 products P = S+ * S-
  PE    : ones-matmul column sums of f (accumulated in PSUM) for the linear
          term; host applies the +-1 weights per column half
Sharding: data-parallel over rows, 1024 rows per core, 8 cores.
"""

import os
import numpy as np

import concourse.bass as bass
import concourse.tile as tile
from concourse import bacc, mybir
from concourse.bass_utils import run_bass_kernel_spmd

B = 8192
N = 2048
HALF = N // 2
NCORES = 8
P = 128
ROWS_PER_CORE = B // NCORES          # 1024
NTILES_FULL = ROWS_PER_CORE // P     # 8

AF = mybir.ActivationFunctionType
OP = mybir.AluOpType
FP32 = mybir.dt.float32
BF16 = mybir.dt.bfloat16


def build_program(ntiles=NTILES_FULL):
    """Build the SPMD single-core Bass program.

    Input : f_shard  [ntiles*128, 2048] fp32
    Output: partials [128, 1] fp32  -- per-partition sum of ln(S+ * S-)
            colsums  [1, 2048] fp32 -- per-column sum of f over all rows
    """
    assert ntiles % 2 == 0
    npairs = ntiles // 2
    nc = bacc.Bacc("TRN2", target_bir_lowering=False, debug=False,
                   num_devices=NCORES)
    f_in = nc.dram_tensor("f_shard", [ntiles * P, N], FP32, kind="ExternalInput")
    out_ln = nc.dram_tensor("partials", [P, 4], FP32, kind="ExternalOutput")
    out_cs = nc.dram_tensor("colsums", [1, N], FP32, kind="ExternalOutput")

    # DRAM view for pair loads: fbuf[p, i*2048+c] = f[pair*256 + i*128 + p, c]
    f_pairs = f_in.rearrange("(b i p) c -> b p i c", i=2, p=P)

    with tile.TileContext(nc) as tc:
        with (
            tc.tile_pool(name="io", bufs=npairs + 1) as io_pool,
            tc.tile_pool(name="ework", bufs=3) as e_pool,
            tc.tile_pool(name="big", bufs=1) as big_pool,
            tc.tile_pool(name="psum", bufs=1, space="PSUM") as psum_pool,
        ):
            # S layout: per tile a 2048-slot: [S+ (1024) | S- (1024)]
            Sbig = big_pool.tile([P, ntiles * N], BF16)
            lndump = big_pool.tile([P, ntiles * N], BF16)
            res_ln = big_pool.tile([P, 4], FP32)
            ones = big_pool.tile([P, 1], FP32)
            cs_sb = big_pool.tile([1, N], FP32)
            nc.gpsimd.memset(ones[:, :], 1.0)
            pts = [psum_pool.tile([1, 512], FP32, name=f"pt{c}", tag=f"pt{c}")
                   for c in range(4)]

            for pr in range(npairs):
                fbuf = io_pool.tile([P, 2 * N], FP32, tag="fbuf")
                if pr == 0:
                    # split the first load so ACT can start ~3us earlier, and
                    # issue it from the GPSIMD SWDGE queue which boots sooner
                    fv = fbuf[:, :].rearrange("p (i c) -> p i c", i=2)
                    nc.gpsimd.dma_start(fv[:, 0, :], f_pairs[0][:, 0, :])
                    nc.gpsimd.dma_start(fv[:, 1, :], f_pairs[0][:, 1, :])
                else:
                    nc.sync.dma_start(
                        fbuf[:, :].rearrange("p (i c) -> p i c", i=2),
                        f_pairs[pr])

                epos = e_pool.tile([P, 2 * N], BF16, tag="epos")
                eneg = e_pool.tile([P, 2 * N], BF16, tag="eneg")
                if pr == 0:
                    for i in range(2):
                        sl = slice(i * N, (i + 1) * N)
                        nc.scalar.activation(epos[:, sl], fbuf[:, sl], AF.Exp)
                        nc.scalar.activation(eneg[:, sl], fbuf[:, sl], AF.Exp,
                                             scale=-1.0)
                else:
                    nc.scalar.activation(epos[:, :], fbuf[:, :], AF.Exp)
                    nc.scalar.activation(eneg[:, :], fbuf[:, :], AF.Exp,
                                         scale=-1.0)

                for i in range(2):
                    t = 2 * pr + i
                    c0 = i * N               # tile column base in pair bufs
                    w0 = t * N               # tile slot base in Sbig
                    # center-out window sums, both signs (output order is
                    # reversed windows -- irrelevant under the final sum)
                    nc.vector.tensor_tensor_scan(
                        Sbig[:, w0:w0 + HALF],
                        epos[:, c0 + HALF - 1:c0 - 1 if c0 else None:-1],
                        epos[:, c0 + HALF:c0 + N],
                        0.0, OP.add, OP.add)
                    nc.vector.tensor_tensor_scan(
                        Sbig[:, w0 + HALF:w0 + N],
                        eneg[:, c0 + HALF - 1:c0 - 1 if c0 else None:-1],
                        eneg[:, c0 + HALF:c0 + N],
                        0.0, OP.add, OP.add)
                    # linear-term column sums on PE, accumulated across tiles
                    for c in range(4):
                        nc.tensor.matmul(
                            pts[c][:, :],
                            ones[:, :],
                            fbuf[:, c0 + c * 512:c0 + (c + 1) * 512],
                            start=(t == 0), stop=(t == ntiles - 1))

            # sum_i ln S_i, graduated split: early chunks overlap the
            # remaining scans; only a single-tile Ln sits in the tail
            cuts = [0, 4 * N, 6 * N, 7 * N, 8 * N]
            if ntiles != 8:
                cuts = [0, ntiles * N // 2, ntiles * N]
            for j in range(len(cuts) - 1):
                nc.scalar.activation(lndump[:, cuts[j]:cuts[j + 1]],
                                     Sbig[:, cuts[j]:cuts[j + 1]], AF.Ln,
                                     accum_out=res_ln[:, j:j + 1])
            for c in range(4):
                nc.vector.tensor_copy(cs_sb[:, c * 512:(c + 1) * 512],
                                      pts[c][:, :])
            nc.sync.dma_start(out_ln[:, :], res_ln[:, :])
            nc.sync.dma_start(out_cs[:, :], cs_sb[:, :])

    nc.compile()
    return nc


_last_results = None  # test.py reads exec_time_ns from here


def kernel(f, num_stocks):
    global _last_results
    f = np.ascontiguousarray(np.asarray(f), dtype=np.float32)
    assert f.shape == (B, N) and int(num_stocks) == N

    nc = build_program()
    in_maps = [
        {"f_shard": f[c * ROWS_PER_CORE:(c + 1) * ROWS_PER_CORE]}
        for c in range(NCORES)
    ]
    res = run_bass_kernel_spmd(
        nc, in_maps, core_ids=list(range(NCORES)),
        trace=bool(int(os.environ.get("KERNEL_TRACE", "0"))),
    )
    _last_results = res

    total = 0.0
    for r in res.results:
        total += r["partials"].astype(np.float64).sum()
        cs = r["colsums"].astype(np.float64)[0]
        total += cs[HALF:].sum() - cs[:HALF].sum()
    return np.float32(total / B)


# revision 16
# speedup vs baseline: 1.1957x; 1.1957x over previous
"""Trainium2 Bass kernel for nn_Closs_58978490909000.

Reference computes, per row r of f (8192 x 2048), with half = 1024:
    l_r = sum(f[r, half:]) - sum(f[r, :half])
          + sum_{i=0}^{half-1} [ logsumexp(f[r, i:N-i]) + logsumexp(-f[r, i:N-i]) ]
and returns mean_r l_r.

Algorithm (O(N) per row instead of O(N^2/2)):
  The windows are nested, so windowed sums of e = exp(+-f) obey a center-out
  additive recurrence.  A single DVE tensor_tensor_scan per row-tile computes
  all 1024 window sums:
      state_t = (e[1023-t] + state_{t-1}) + e[1024+t]     (t = 0..1023)
  (data0 = low half read with stride -1, data1 = high half forward), giving
  state_t = S_{1023-t}.  Then
      sum_i [ln S+_i + ln S-_i] = sum_i ln(S+_i * S-_i)
  so one elementwise product (on GPSIMD) halves the ACT ln work, and one Ln
  activation with fused accum_out produces the per-partition total.  No
  max-subtraction is needed: f ~ N(0,1) so exp() stays in [e^-6, e^6].

Engine budget per core (8 row-tiles of 128 rows x 2048):
  ACT   : 8 pair-batched exps ([128,4096] fp32->bf16) + 1 giant Ln [128,8192]
  DVE   : 16 scans (bf16, ~2.26us each) -- nothing else
  POOL  : 8 bf16 products P = S+ * S-
  PE    : ones-matmul column sums of f (accumulated in PSUM) for the linear
          term; host applies the +-1 weights per column half
Sharding: data-parallel over rows, 1024 rows per core, 8 cores.
"""

import os
import numpy as np

import concourse.bass as bass
import concourse.tile as tile
from concourse import bacc, mybir
from concourse.bass_utils import run_bass_kernel_spmd

B = 8192
N = 2048
HALF = N // 2
NCORES = 8
P = 128
ROWS_PER_CORE = B // NCORES          # 1024
NTILES_FULL = ROWS_PER_CORE // P     # 8

AF = mybir.ActivationFunctionType
OP = mybir.AluOpType
FP32 = mybir.dt.float32
BF16 = mybir.dt.bfloat16


def build_program(ntiles=NTILES_FULL):
    """Build the SPMD single-core Bass program.

    Input : f_shard  [ntiles*128, 2048] fp32
    Output: partials [128, 1] fp32  -- per-partition sum of ln(S+ * S-)
            colsums  [1, 2048] fp32 -- per-column sum of f over all rows
    """
    assert ntiles % 2 == 0
    npairs = ntiles // 2
    nc = bacc.Bacc("TRN2", target_bir_lowering=False, debug=False,
                   num_devices=NCORES)
    f_in = nc.dram_tensor("f_shard", [ntiles * P, N], FP32, kind="ExternalInput")
    n_ln_chunks = 4 if ntiles == 8 else 2
    out_ln = nc.dram_tensor("partials", [P, n_ln_chunks], FP32,
                            kind="ExternalOutput")
    out_cs = nc.dram_tensor("colsums", [1, N], FP32, kind="ExternalOutput")

    # DRAM view for pair loads: fbuf[p, i*2048+c] = f[pair*256 + i*128 + p, c]
    f_pairs = f_in.rearrange("(b i p) c -> b p i c", i=2, p=P)

    with tile.TileContext(nc) as tc:
        with (
            tc.tile_pool(name="io", bufs=npairs + 1) as io_pool,
            tc.tile_pool(name="ework", bufs=3) as e_pool,
            tc.tile_pool(name="big", bufs=1) as big_pool,
            tc.tile_pool(name="psum", bufs=1, space="PSUM") as psum_pool,
        ):
            # S layout: per tile a 2048-slot: [S+ (1024) | S- (1024)]
            Sbig = big_pool.tile([P, ntiles * N], BF16)
            lndump = big_pool.tile([P, ntiles * N], BF16)
            res_ln = big_pool.tile([P, n_ln_chunks], FP32)
            ones = big_pool.tile([P, 1], FP32)
            cs_sb = big_pool.tile([1, N], FP32)
            nc.gpsimd.memset(ones[:, :], 1.0)
            pts = [psum_pool.tile([1, 512], FP32, name=f"pt{c}", tag=f"pt{c}")
                   for c in range(4)]

            for pr in range(npairs):
                fbuf = io_pool.tile([P, 2 * N], FP32, tag="fbuf")
                if pr == 0:
                    # split the first load so ACT can start ~3us earlier
                    fv = fbuf[:, :].rearrange("p (i c) -> p i c", i=2)
                    nc.sync.dma_start(fv[:, 0, :], f_pairs[0][:, 0, :])
                    nc.sync.dma_start(fv[:, 1, :], f_pairs[0][:, 1, :])
                else:
                    nc.sync.dma_start(
                        fbuf[:, :].rearrange("p (i c) -> p i c", i=2),
                        f_pairs[pr])

                epos = e_pool.tile([P, 2 * N], BF16, tag="epos")
                eneg = e_pool.tile([P, 2 * N], BF16, tag="eneg")
                if pr == 0:
                    for i in range(2):
                        sl = slice(i * N, (i + 1) * N)
                        nc.scalar.activation(epos[:, sl], fbuf[:, sl], AF.Exp)
                        nc.scalar.activation(eneg[:, sl], fbuf[:, sl], AF.Exp,
                                             scale=-1.0)
                else:
                    nc.scalar.activation(epos[:, :], fbuf[:, :], AF.Exp)
                    nc.scalar.activation(eneg[:, :], fbuf[:, :], AF.Exp,
                                         scale=-1.0)

                for i in range(2):
                    t = 2 * pr + i
                    c0 = i * N               # tile column base in pair bufs
                    w0 = t * N               # tile slot base in Sbig
                    # center-out window sums, both signs (output order is
                    # reversed windows -- irrelevant under the final sum)
                    nc.vector.tensor_tensor_scan(
                        Sbig[:, w0:w0 + HALF],
                        epos[:, c0 + HALF - 1:c0 - 1 if c0 else None:-1],
                        epos[:, c0 + HALF:c0 + N],
                        0.0, OP.add, OP.add)
                    nc.vector.tensor_tensor_scan(
                        Sbig[:, w0 + HALF:w0 + N],
                        eneg[:, c0 + HALF - 1:c0 - 1 if c0 else None:-1],
                        eneg[:, c0 + HALF:c0 + N],
                        0.0, OP.add, OP.add)
                    # linear-term column sums on PE, accumulated across tiles
                    for c in range(4):
                        nc.tensor.matmul(
                            pts[c][:, :],
                            ones[:, :],
                            fbuf[:, c0 + c * 512:c0 + (c + 1) * 512],
                            start=(t == 0), stop=(t == ntiles - 1))

            # sum_i ln S_i, graduated split: early chunks overlap the
            # remaining scans; only a single-tile Ln sits in the tail
            cuts = ([0, 4 * N, 6 * N, 7 * N, 8 * N] if ntiles == 8
                    else [0, ntiles * N // 2, ntiles * N])
            for j in range(len(cuts) - 1):
                nc.scalar.activation(lndump[:, cuts[j]:cuts[j + 1]],
                                     Sbig[:, cuts[j]:cuts[j + 1]], AF.Ln,
                                     accum_out=res_ln[:, j:j + 1])
            for c in range(4):
                nc.vector.tensor_copy(cs_sb[:, c * 512:(c + 1) * 512],
                                      pts[c][:, :])
            nc.sync.dma_start(out_ln[:, :], res_ln[:, :])
            nc.sync.dma_start(out_cs[:, :], cs_sb[:, :])

    nc.compile()
    return nc


_last_results = None  # test.py reads exec_time_ns from here


def kernel(f, num_stocks):
    global _last_results
    f = np.ascontiguousarray(np.asarray(f), dtype=np.float32)
    assert f.shape == (B, N) and int(num_stocks) == N

    nc = build_program()
    in_maps = [
        {"f_shard": f[c * ROWS_PER_CORE:(c + 1) * ROWS_PER_CORE]}
        for c in range(NCORES)
    ]
    res = run_bass_kernel_spmd(
        nc, in_maps, core_ids=list(range(NCORES)),
        trace=bool(int(os.environ.get("KERNEL_TRACE", "0"))),
    )
    _last_results = res

    total = 0.0
    for r in res.results:
        total += r["partials"].astype(np.float64).sum()
        cs = r["colsums"].astype(np.float64)[0]
        total += cs[HALF:].sum() - cs[:HALF].sum()
    return np.float32(total / B)
